# revision 22
# baseline (speedup 1.0000x reference)
"""LiteLinear (dense linear + per-token LoRA adapters) on 8 Trainium2 cores.

Sharding: data-parallel over tokens. Each core computes 1024 tokens:
  out = x @ W^T + bias + per-token LoRA delta.

Device kernel (per core), all matmuls in bf16 (fp32 PSUM accumulate;
bf16 gets the fast FWL weight load and halves HBM traffic vs fp32r):
  - Computes out^T [D_OUT x TOK]; host transposes back on assembly.
  - Stationary operand = weight sub-chunk [128d x 128o], moving = x^T
    [128d x 512tok]; x^T is resident in SBUF, relaid out on the host
    partition-major ([p, k, t]) so the whole 8MB loads in 7 DMAs of
    long contiguous rows (DMA issue costs ~0.65us per 128 descriptor
    rows on the issuing engine, so row count is what matters).
  - A_cat^T (the concatenated LoRA down-projections) is prepended to W^T
    as a 33rd output column tile, so h^T = A_cat @ x^T rides the same
    streamed matmul pipeline; its eviction is a DVE multiply with a
    host-built maskT (folds scalings + one-hot) producing hmask^T.
  - W^T is relaid out per o-group, partition-major: one group's whole
    weight block is a single [128 x 32KB-row] DMA. Groups are issued a
    full group ahead of use (DMA issue is gated on the tensor counter
    at its program point, so early program placement = real prefetch).
  - o-groups of [4,4,4,4,4,4,4,4,1] x128 tiles (33 total, first group
    includes the A tile); psum = width x 2 token-halves banks.
  - Per-token LoRA delta enters each out-tile as one extra accumulating
    matmul (lhsT=B_cat chunk, rhs=hmask^T); group 0 evicts the h tile
    (producing hmask) before issuing its own deltas.
  - Bias folded into PSUM->SBUF eviction via per-partition
    tensor_scalar_add. Engine split: sync ring = W stream only,
    scalar ring = x + consts + output stores (all hardware DGE).
"""

import ml_dtypes
import numpy as np

import sys

if "/opt/trn_rl_repo" not in sys.path:
    sys.path.insert(0, "/opt/trn_rl_repo")

import concourse.bass as bass
import concourse.mybir as mybir
import concourse.tile as tile
from concourse import bacc
from concourse.bass_utils import run_bass_kernel_spmd

N_TOK = 8192
D_IN = 4096
D_OUT = 4096
N_ADAPTERS = 8
RANK = 16
AR = N_ADAPTERS * RANK  # 128
N_CORES = 8
TOK = N_TOK // N_CORES  # 1024 tokens per core

P = 128            # partitions
FREE = 512         # matmul moving free dim (== 1 PSUM bank of fp32)
KC = D_IN // P     # 32 contraction chunks
TH = TOK // FREE   # 2 token halves
NO = D_OUT // P + 1  # 33 o128-tiles incl. the A tile (index 0)
GROUPS = [4] + [3] * 7 + [2] * 4  # o128-tiles per group (sum 33)
# Wide first group: halves x-consumption rate while the startup stream
# lands. <=6-bank groups after: the 8-bank PSUM rotation hands each new
# group banks whose evictions completed early in the previous burst.
LASTG = len(GROUPS) - 1
LX = KC * TOK      # x row length per partition
LW = KC * NO * P   # w row length per partition

F32 = mybir.dt.float32
BF16 = mybir.dt.bfloat16
NPBF16 = ml_dtypes.bfloat16

_CACHE = {}


def _build_nc():
    nc = bacc.Bacc(None, target_bir_lowering=False, debug=True)

    # partition-major resident x: xsw[p, k*TOK + t] = x^T[k*128+p, t]
    xsw = nc.dram_tensor("xsw", [P, LX], BF16, kind="ExternalInput")
    # group-major W: per partition p, [g][k][c_g] contiguous blocks
    wTr = nc.dram_tensor("wTr", [P, LW], BF16, kind="ExternalInput")
    bcat = nc.dram_tensor("bcat", [AR, D_OUT], BF16, kind="ExternalInput")
    maskT = nc.dram_tensor("maskT", [AR, TOK], F32, kind="ExternalInput")
    biasr = nc.dram_tensor("biasr", [P, D_OUT // P], F32, kind="ExternalInput")
    outT = nc.dram_tensor("outT", [D_OUT, TOK], F32, kind="ExternalOutput")

    goffs = []
    o0 = 0
    for wdt in GROUPS:
        goffs.append(KC * o0 * P)
        o0 += wdt

    def w_src(g, sub, blk):
        return bass.AP(tensor=wTr[:].tensor, offset=goffs[g] + sub,
                       ap=[[LW, P], [1, blk]])

    def x_src(k0, nk):
        return bass.AP(tensor=xsw[:].tensor, offset=k0 * TOK,
                       ap=[[LX, P], [1, nk * TOK]])

    with tile.TileContext(nc) as tc:
        with (
            tc.tile_pool(name="xpool", bufs=1) as xpool,
            tc.tile_pool(name="const", bufs=1) as const,
            tc.tile_pool(name="wpool", bufs=3) as wpool,
            tc.tile_pool(name="opool", bufs=3) as opool,
            tc.tile_pool(name="psum", bufs=8, space="PSUM") as psum,
        ):
            hmask = const.tile([P, TOK], BF16, tag="hmask")
            biasr_sb = const.tile([P, D_OUT // P], F32, tag="biasr")
            maskT_sb = const.tile([P, TOK], F32, tag="maskT")
            bcat_sb = const.tile([P, D_OUT], BF16, tag="bcat")

            xall = xpool.tile([P, LX], BF16, tag="xall")

            wtiles = {}

            def issue_w(g, pieces=None):
                width = GROUPS[g]
                blk = KC * width * P
                t = wpool.tile([P, blk], BF16, tag="wt", name=f"wt{g}")
                for k0, nk in (pieces or [(0, KC)]):
                    nc.sync.dma_start(
                        out=t[:, k0 * width * P:(k0 + nk) * width * P],
                        in_=w_src(g, k0 * width * P, nk * width * P))
                wtiles[g] = t

            def issue_x(eng, k0, nk):
                eng.dma_start(out=xall[:, k0 * TOK:(k0 + nk) * TOK],
                              in_=x_src(k0, nk))

            # Both rings feed the startup x stream in need-order
            # alternation, with the early W groups slotted between on
            # sync; consts ride scalar. All prologue issues are gated
            # at tensor-count 0, so ring order == arrival order.
            issue_w(0, pieces=[(0, 1), (1, 7), (8, 12), (20, 12)])  # sync
            issue_x(nc.scalar, 0, 1)
            issue_x(nc.sync, 1, 1)
            issue_x(nc.scalar, 2, 2)
            issue_x(nc.scalar, 4, 4)
            issue_x(nc.sync, 8, 4)
            issue_w(1, pieces=[(0, 16), (16, 16)])          # sync
            issue_x(nc.scalar, 12, 4)
            issue_x(nc.sync, 16, 4)
            issue_w(2)                                      # sync
            issue_x(nc.scalar, 20, 4)
            issue_x(nc.sync, 24, 4)
            issue_x(nc.scalar, 28, 4)
            nc.scalar.dma_start(out=biasr_sb[:], in_=biasr[:, :])
            nc.scalar.dma_start(out=maskT_sb[:], in_=maskT[:, :])
            nc.scalar.dma_start(out=bcat_sb[:], in_=bcat[:, :])

            def base_loop(g, width, pg):
                """32 k-chunks of base matmuls for one o-group."""
                for k in range(KC):
                    if k == 8 and g + 2 <= LASTG:
                        issue_w(g + 2)
                    for j in range(width):
                        wt = wtiles[g]
                        lhsT = wt[:, (k * width + j) * P:
                                  (k * width + j + 1) * P]
                        for th in range(TH):
                            nc.tensor.matmul(
                                pg[j * TH + th][:],
                                lhsT,
                                xall[:, k * TOK + th * FREE:
                                     k * TOK + (th + 1) * FREE],
                                start=(k == 0),
                                stop=(k == KC - 1 and g == 0 and j == 0),
                            )

            for g, width in enumerate(GROUPS):
                pg = [
                    psum.tile([P, FREE], F32, tag="ps", name=f"pg{g}_{i}")
                    for i in range(width * TH)
                ]
                base_loop(g, width, pg)

                j0 = 0
                if g == 0:
                    # evict the A tile -> hmask (scaled, masked); no delta
                    for th in range(TH):
                        tsl = slice(th * FREE, (th + 1) * FREE)
                        nc.vector.tensor_mul(
                            hmask[:, tsl], pg[th][:], maskT_sb[:, tsl])
                    j0 = 1
                # per-j: delta matmul, then evict+bias, then out DMA
                for j in range(j0, width):
                    om = (goffs[g] // (KC * P)) + j - 1
                    for th in range(TH):
                        tsl = slice(th * FREE, (th + 1) * FREE)
                        nc.tensor.matmul(
                            pg[j * TH + th][:],
                            bcat_sb[:, om * P:(om + 1) * P],
                            hmask[:, tsl],
                            start=False, stop=True,
                        )
                    ob = opool.tile([P, TOK], F32, tag="ob", name=f"ob_{om}")
                    for th in range(TH):
                        tsl = slice(th * FREE, (th + 1) * FREE)
                        nc.vector.tensor_scalar_add(
                            ob[:, tsl], pg[j * TH + th][:],
                            biasr_sb[:, om:om + 1],
                        )
                        nc.scalar.dma_start(
                            out=outT[om * P:(om + 1) * P, tsl],
                            in_=ob[:, tsl],
                        )

    nc.compile()
    return nc


def _prep_inputs(x, weight, bias, lora_a, lora_b, scalings, lora_mapping):
    x = np.ascontiguousarray(x, dtype=np.float32)
    weight = np.ascontiguousarray(weight, dtype=np.float32)
    bias = np.ascontiguousarray(bias, dtype=np.float32)
    lora_a = np.ascontiguousarray(lora_a, dtype=np.float32)
    lora_b = np.ascontiguousarray(lora_b, dtype=np.float32)
    scalings = np.ascontiguousarray(scalings, dtype=np.float32)
    lora_mapping = np.asarray(lora_mapping)

    aT = lora_a.transpose(2, 0, 1).reshape(D_IN, AR)                 # [D_IN,(a r)]
    awT = np.concatenate([aT, weight.T], axis=1).astype(NPBF16)      # [D_IN, NO*P]
    # group-major, partition-major W: per p, [g][k][c_g] contiguous
    w3 = awT.reshape(KC, P, NO * P)                                  # [k,p,o]
    blocks = []
    o0 = 0
    for wdt in GROUPS:
        blk = w3[:, :, o0:o0 + wdt * P]                              # [k,p,w]
        blocks.append(blk.transpose(1, 0, 2).reshape(P, KC * wdt * P))
        o0 += wdt * P
    wTr = np.ascontiguousarray(np.concatenate(blocks, axis=1))       # [P, LW]

    bcat = np.ascontiguousarray(
        lora_b.transpose(0, 2, 1).reshape(AR, D_OUT).astype(NPBF16))  # [(a r), D_OUT]
    # biasr[p, m] = bias[m*128 + p]
    biasr = np.ascontiguousarray(bias.reshape(D_OUT // P, P).T)      # [P, 32]
    # maskT[(a r), n] = scalings[a] * (lora_mapping[n] == a+1)
    ids = np.arange(1, N_ADAPTERS + 1, dtype=lora_mapping.dtype)
    onehot = (lora_mapping[None, :] == ids[:, None]).astype(np.float32)  # [A, N]
    maskT = (onehot * scalings[:, None]).repeat(RANK, axis=0)        # [(a r), N]
    maskT = np.ascontiguousarray(maskT)

    xbf = x.astype(NPBF16)                                           # [N_TOK, D_IN]
    in_maps = []
    for c in range(N_CORES):
        tsl = slice(c * TOK, (c + 1) * TOK)
        # xsw[p, k*TOK+t] = x[c*TOK+t, k*128+p]
        xsw = np.ascontiguousarray(
            xbf[tsl].T.reshape(KC, P, TOK).transpose(1, 0, 2).reshape(P, LX))
        in_maps.append({
            "xsw": xsw,
            "wTr": wTr,
            "bcat": bcat,
            "maskT": np.ascontiguousarray(maskT[:, tsl]),
            "biasr": biasr,
        })
    return in_maps


def run(inputs, trace=False):
    if "nc" not in _CACHE:
        _CACHE["nc"] = _build_nc()
    nc = _CACHE["nc"]
    in_maps = _prep_inputs(**inputs)
    res = run_bass_kernel_spmd(
        nc, in_maps, list(range(N_CORES)), trace=trace,
    )
    out = np.concatenate(
        [np.ascontiguousarray(r["outT"].T) for r in res.results], axis=0
    )
    return out, res


def kernel(**inputs) -> np.ndarray:
    out, _ = run(inputs, trace=False)
    return out
